# revision 15
# baseline (speedup 1.0000x reference)
"""Trainium2 Bass kernel for nn_CurvStdDist (retrieval_knn).

Reference computation (per batch b, per cloud):
  x: (n,3) points, nrm: (n,3) unit normals, k=16
  idx   = 16 nearest neighbors of each point (excluding self, by squared L2)
  v     = x[idx] - x[:,None]; vhat = v / clip(||v||, 1e-12)
  kappa = mean_k |vhat . nrm|                      (n,)
  std   = std(kappa[idx], ddof=1)                  (n,)
Final: dist = mean_b ||ori_std[b] - adv_std[b] + 1e-6||_2

Sharding: 8 cores = 4 batches x 2 clouds (ori/adv); each core runs the
full n=4096 KNN pipeline for one (batch, cloud); host combines the 8
std vectors into the scalar.

Device algorithm per core, pass 1 (per 128-row tile):
  - -d2 row-tile [128,4096] via K=5 fp32 matmul into 8 psum banks:
      -d2[i,j] = [2x_i, -|x_i|^2, -1] . [x_j, 1, |x_j|^2]
    plus a second matmul adding -1e38*I on the tile's diagonal bank
    (self-exclusion). ACT engine evacuates banks to SBUF.
  - ONE scalar_tensor_tensor packs the column index into the low 12
    mantissa bits of each -d2 value: pk = (bits & 0xFFFFF000) | j.
    Quantization (4096 ulp) only perturbs near-ties among neighbors;
    far below the typical 16th/17th-neighbor distance gap.
  - top-16 per row via two-stage selection on packed values: max8 per
    512-column chunk (8 candidates each, exact unless >8 of the true
    top-16 share one chunk: P ~ 3e-5 per row), then max8 /
    match_replace / max8 over the 64 candidates. Neighbor indices drop
    out of the packed low bits with one AND — no max_index passes.
  - the 16th packed value rounded up one quantization step becomes a
    per-row threshold t_adj: S[i,j] >= t_adj[i] selects exactly the
    chosen 16 neighbors (plus rare quantization ties).
  - 16 per-slot indirect (SWDGE) gathers fetch neighbor coords (the HW
    indirect DMA consumes one index per partition); kappa via DVE/ACT
    elementwise ops, software-pipelined one tile behind the top-k.
    kappa accumulates in SBUF (scaled by 16), stored to DRAM once.
Pass 2 (gather-free phase B): broadcast kappa across partitions with a
ones-column matmul (kb[i,j] = 16*kappa_j), k2b = Square(kb) on ACT,
re-run the (deterministic, bit-identical) -d2 matmul per tile, and per
moment one fused scalar_tensor_tensor computes
  s{1,2}[i] = sum_j (S[i,j] >= t_adj[i]) * k{1,2}b[i,j]
via its free-dim accumulator. std = sqrt(max(s2-s1^2/16,0)/(15*256)).
The kappa DRAM round-trip is ordered by an explicit store->load dep
instead of an all-engine barrier so successive reps pipeline.
"""

import numpy as np

N = 4096          # points per cloud
P = 128           # partitions
T = N // P        # 32 row tiles
K = 16            # neighbors
BANK = 512        # psum bank width (f32)
NBANK = N // BANK
DIAG_NEG = -1.0e38   # added on the diagonal (self distance)
FILL_NEG = -3.0e38   # match_replace fill
CBIAS = 1024.0       # count-bias folded into the kappa broadcast table

_PROG_CACHE = {}


def _build_program(stage="full", reps=1):
    """Build + compile the single-core Bass program (shared by all 8 cores).

    stage: "mm" | "topk" | "idx" | "gather" | "kappa" | "std1" | "full"
    reps: repeat the whole pipeline (timing harness: marginal wall per rep).
    """
    import concourse.bacc as bacc
    import concourse.bass as bass
    import concourse.mybir as mybir
    import concourse.tile as tile
    from concourse.tile_autobufs import add_dep_helper

    dt = mybir.dt
    AF = mybir.ActivationFunctionType
    Alu = mybir.AluOpType

    nc = bacc.Bacc("TRN2", target_bir_lowering=False, debug=False)

    lhsT5 = nc.dram_tensor("lhsT5", [5, N], dt.float32, kind="ExternalInput")
    rhs5 = nc.dram_tensor("rhs5", [5, N], dt.float32, kind="ExternalInput")
    xyz = nc.dram_tensor("xyz", [N, 3], dt.float32, kind="ExternalInput")
    nrm = nc.dram_tensor("nrm", [N, 3], dt.float32, kind="ExternalInput")
    eye = nc.dram_tensor("eye", [P, P], dt.float32, kind="ExternalInput")
    # -1e38*I at columns 384:512 of a zero [P, 896]; slicing [384-off : 896-off]
    # yields a [P, 512] bank-row with the negative diagonal at columns off:off+P
    negpad = nc.dram_tensor("negpad", [P, 896], dt.float32, kind="ExternalInput")
    # iota[p, j] = j (uint32) — OR'd into the low mantissa bits
    iota = nc.dram_tensor("iota", [P, N], dt.uint32, kind="ExternalInput")
    # masks[:,0]=0xFFFFF000 (pack); [:,1:17]=0xFFF (extract); [:,17]=0x1000
    masks = nc.dram_tensor("masks", [P, 18], dt.uint32, kind="ExternalInput")
    ones1 = nc.dram_tensor("ones1", [1, P], dt.float32, kind="ExternalInput")
    fbias = nc.dram_tensor("fbias", [P, 1], dt.float32, kind="ExternalInput")
    kap_d = nc.dram_tensor("kappa", [N, 1], dt.float32, kind="ExternalOutput")
    std_d = nc.dram_tensor("std", [N, 1], dt.float32, kind="ExternalOutput")

    def bcast_mid(ap, k):
        # [P, (1,) c] -> [P, k, c] with a stride-0 middle dim
        return bass.AP(ap.tensor, ap.offset, [ap.ap[0], [0, k], ap.ap[-1]])

    with tile.TileContext(nc) as tc:
        with (
            tc.tile_pool(name="const", bufs=1) as constp,
            tc.tile_pool(name="srow", bufs=2) as sp,
            tc.tile_pool(name="psum", bufs=NBANK, space="PSUM") as pp,
            tc.tile_pool(name="small", bufs=4) as smp,
            tc.tile_pool(name="idxp", bufs=1) as idxp,
        ):
            lh = constp.tile_from(lhsT5.ap())
            rh = constp.tile_from(rhs5.ap())
            ey = constp.tile_from(eye.ap())
            npd = constp.tile_from(negpad.ap())
            io = constp.tile_from(iota.ap())
            mk = constp.tile_from(masks.ap())
            on1 = constp.tile_from(ones1.ap())
            fb = constp.tile_from(fbias.ap())
            idx_all = idxp.tile([P, T * K], dt.uint32)
            kap_sb = idxp.tile([P, T, 1], dt.float32)
            t_all = idxp.tile([P, T], dt.float32)
            s1_all = idxp.tile([P, T], dt.float32)
            s2_all = idxp.tile([P, T], dt.float32)
            kb = idxp.tile([P, N], dt.float32)
            kf = idxp.tile([1, N, 1], dt.float32)
            k2b = idxp.tile([P, N], dt.float32)
            junk = idxp.tile([P, N], dt.float32)
            # all tiles' own coords/normals in one DMA: [p, t, c] <- row t*P+p
            xi_all = constp.tile([P, T, 3], dt.float32)
            nc.sync.dma_start(
                xi_all[:], xyz.ap().rearrange("(t p) c -> p t c", p=P)
            )
            ni_all = constp.tile([P, T, 3], dt.float32)
            nc.sync.dma_start(
                ni_all[:], nrm.ap().rearrange("(t p) c -> p t c", p=P)
            )

            def kappa_math(t, nn):
                # nn: [P, K*3] gathered neighbor coords for tile t
                xi = xi_all[:, t : t + 1, :]
                ni = ni_all[:, t : t + 1, :]
                nn3 = nn[:].rearrange("p (k c) -> p k c", c=3)
                v = smp.tile([P, K * 3], dt.float32, tag="v")
                v3 = v[:].rearrange("p (k c) -> p k c", c=3)
                nc.vector.tensor_tensor(
                    out=v3, in0=nn3, in1=bcast_mid(xi, K), op=Alu.subtract
                )
                vn = smp.tile([P, K * 3], dt.float32, tag="vn")
                vn3 = vn[:].rearrange("p (k c) -> p k c", c=3)
                nc.vector.tensor_tensor(
                    out=vn3, in0=v3, in1=bcast_mid(ni, K), op=Alu.mult
                )
                dot = smp.tile([P, K], dt.float32, tag="dot")
                nc.vector.tensor_reduce(
                    dot[:], vn3, axis=mybir.AxisListType.X, op=Alu.add
                )
                v2 = smp.tile([P, K * 3], dt.float32, tag="v2")
                v23 = v2[:].rearrange("p (k c) -> p k c", c=3)
                nc.vector.tensor_tensor(out=v23, in0=v3, in1=v3, op=Alu.mult)
                n2 = smp.tile([P, K], dt.float32, tag="n2")
                nc.vector.tensor_reduce(
                    n2[:], v23, axis=mybir.AxisListType.X, op=Alu.add
                )
                # clip ||v||^2 at 1e-24 (reference clips ||v|| at 1e-12)
                nc.vector.tensor_scalar_max(n2[:], n2[:], 1e-24)
                ri = smp.tile([P, K], dt.float32, tag="ri")
                nc.vector.reciprocal(ri[:], n2[:])
                rs = smp.tile([P, K], dt.float32, tag="rs")
                nc.scalar.activation(rs[:], ri[:], AF.Sqrt)
                sc = smp.tile([P, K], dt.float32, tag="sc")
                nc.vector.tensor_tensor(out=sc[:], in0=dot[:], in1=rs[:], op=Alu.mult)
                nc.vector.tensor_reduce(
                    kap_sb[:, t : t + 1, 0],
                    sc[:],
                    axis=mybir.AxisListType.X,
                    op=Alu.add,
                    apply_absolute_value=True,
                )  # = 16 * kappa

            def dist_tile(t, Sq):
                """Emit the 9 matmuls for row tile t into psum and ACT-copy
                the 8 banks into SBUF tile Sq."""
                bd, off = (t * P) // BANK, (t * P) % BANK
                for b in range(NBANK):
                    ps = pp.tile([P, BANK], dt.float32, tag="ps")
                    nc.tensor.matmul(
                        out=ps[:],
                        lhsT=lh[:, t * P : (t + 1) * P],
                        rhs=rh[:, b * BANK : (b + 1) * BANK],
                        start=True,
                        stop=(b != bd),
                    )
                    if b == bd:
                        nc.tensor.matmul(
                            out=ps[:],
                            lhsT=ey[:],
                            rhs=npd[:, 384 - off : 896 - off],
                            start=False,
                            stop=True,
                        )
                    nc.scalar.copy(Sq[:, b * BANK : (b + 1) * BANK], ps[:])

            for _rep in range(reps):
                # ---------------- pass 1: knn + kappa ----------------
                nn_tiles = {}
                for t in range(T):
                    S = sp.tile([P, N], dt.float32, tag="S")
                    Su = S[:].bitcast(dt.uint32)
                    dist_tile(t, S)

                    if stage == "mm":
                        chk = smp.tile([P, 1], dt.float32, tag="chk")
                        nc.vector.tensor_reduce(
                            chk[:], S[:], axis=mybir.AxisListType.X, op=Alu.max
                        )
                        nc.sync.dma_start(std_d.ap()[t * P : (t + 1) * P, :], chk[:])
                        continue

                    # pack column index into low 12 bits (one full-width op)
                    nc.vector.scalar_tensor_tensor(
                        out=Su,
                        in0=Su,
                        scalar=mk[:, 0:1],
                        in1=io[:],
                        op0=Alu.bitwise_and,
                        op1=Alu.bitwise_or,
                    )
                    # top-8 of each 512 chunk -> 64 packed candidates
                    cand = smp.tile([P, 64], dt.float32, tag="cand")
                    for b in range(NBANK):
                        nc.vector.max(
                            cand[:, 8 * b : 8 * b + 8],
                            S[:, b * BANK : (b + 1) * BANK],
                        )
                    # stage 2: top-16 of the 64 candidates
                    pk = smp.tile([P, 16], dt.float32, tag="pk")
                    nc.vector.max(pk[:, 0:8], cand[:])
                    nc.vector.match_replace(cand[:], pk[:, 0:8], cand[:], FILL_NEG)
                    nc.vector.max(pk[:, 8:16], cand[:])
                    # t_adj = (16th packed value & ~0xFFF) + 0x1000: accepts
                    # exactly the packed-selected 16 (+ rare quant ties)
                    tau = smp.tile([P, 1], dt.uint32, tag="tau")
                    nc.vector.tensor_tensor(
                        out=tau[:], in0=pk[:, 15:16].bitcast(dt.uint32),
                        in1=mk[:, 0:1], op=Alu.bitwise_and,
                    )
                    nc.vector.tensor_tensor(
                        out=t_all[:, t : t + 1].bitcast(dt.uint32),
                        in0=tau[:], in1=mk[:, 17:18], op=Alu.add,
                    )
                    # neighbor indices = packed low 12 bits
                    nc.vector.tensor_tensor(
                        out=idx_all[:, t * K : (t + 1) * K],
                        in0=pk[:].bitcast(dt.uint32),
                        in1=mk[:, 1:17],
                        op=Alu.bitwise_and,
                    )

                    if stage == "topk":
                        chk = smp.tile([P, 1], dt.float32, tag="chk")
                        nc.vector.tensor_reduce(
                            chk[:], pk[:], axis=mybir.AxisListType.X, op=Alu.max
                        )
                        nc.sync.dma_start(std_d.ap()[t * P : (t + 1) * P, :], chk[:])
                        continue
                    if stage == "idx":
                        # dump tiles 0-1 neighbor indices (raw bits) into std
                        if t == 1:
                            nc.sync.dma_start(
                                std_d.ap().rearrange("(a p) c -> p a c", p=P),
                                idx_all[:, 0:32]
                                .bitcast(dt.float32)
                                .rearrange("p (a c) -> p a c", c=1),
                            )
                        continue

                    # gather 16 neighbor coords per point. HW indirect DMA
                    # takes ONE index per partition (contiguous run per
                    # index), so issue one gather per neighbor slot.
                    nn = smp.tile([P, K * 3], dt.float32, tag="nn")
                    for k in range(K):
                        nc.gpsimd.indirect_dma_start(
                            out=nn[:, 3 * k : 3 * k + 3],
                            out_offset=None,
                            in_=xyz.ap(),
                            in_offset=bass.IndirectOffsetOnAxis(
                                ap=idx_all[:, t * K + k : t * K + k + 1], axis=0
                            ),
                        )
                    nn_tiles[t] = nn
                    if stage == "gather":
                        if t == 0:
                            nc.sync.dma_start(
                                std_d.ap().rearrange("(a p) c -> p a c", p=P),
                                nn[:, 0:32].rearrange("p (a c) -> p a c", c=1),
                            )
                        continue

                    # kappa for the PREVIOUS tile: its gather latency hides
                    # under this tile's top-k DVE work
                    if t - 1 in nn_tiles:
                        kappa_math(t - 1, nn_tiles.pop(t - 1))

                if stage in ("mm", "topk", "idx", "gather"):
                    nn_tiles.clear()
                    continue
                for t in sorted(nn_tiles):
                    kappa_math(t, nn_tiles.pop(t))
                kap_store = nc.sync.dma_start(
                    kap_d.ap().rearrange("(t p) c -> p t c", p=P), kap_sb[:]
                )
                if stage == "kappa":
                    continue

                # ------- pass 2: gather-free neighbor-kappa std -------
                # broadcast kappa across partitions: kb[i, j] = 16*kappa_j
                kap_load = nc.sync.dma_start(
                    kf[:], kap_d.ap().rearrange("(a n) c -> a n c", a=1)
                )
                add_dep_helper(
                    kap_load.ins, kap_store.ins, sync=True,
                    reason="kappa DRAM store->load",
                )
                for b in range(NBANK):
                    ps = pp.tile([P, BANK], dt.float32, tag="ps")
                    nc.tensor.matmul(
                        out=ps[:],
                        lhsT=on1[:],
                        rhs=kf[0:1, b * BANK : (b + 1) * BANK, 0],
                        start=True,
                        stop=True,
                    )
                    # kb = 16*kappa + CBIAS: the s1 accumulator then returns
                    # CBIAS*count + s1, giving the exact selected count for
                    # free (count ~ 16, s1 <= 272 << CBIAS)
                    nc.scalar.activation(
                        kb[:, b * BANK : (b + 1) * BANK], ps[:],
                        AF.Identity, bias=fb[:, 0:1],
                    )
                    nc.scalar.activation(
                        k2b[:, b * BANK : (b + 1) * BANK], ps[:], AF.Square
                    )

                for t in range(T):
                    S2 = sp.tile([P, N], dt.float32, tag="S")
                    dist_tile(t, S2)
                    nc.vector.scalar_tensor_tensor(
                        out=junk[:],
                        in0=S2[:],
                        scalar=t_all[:, t : t + 1],
                        in1=kb[:],
                        op0=Alu.is_ge,
                        op1=Alu.mult,
                        accum_out=s1_all[:, t : t + 1],
                    )
                    nc.vector.scalar_tensor_tensor(
                        out=junk[:],
                        in0=S2[:],
                        scalar=t_all[:, t : t + 1],
                        in1=k2b[:],
                        op0=Alu.is_ge,
                        op1=Alu.mult,
                        accum_out=s2_all[:, t : t + 1],
                    )

                if stage == "std1":
                    nc.sync.dma_start(
                        std_d.ap().rearrange("(t p) c -> p t c", p=P),
                        s1_all[:].rearrange("p (t c) -> p t c", c=1),
                    )
                    continue
                # decode count c and s1 from s1p = CBIAS*c + s1, then
                # std = sqrt(max(s2 - s1^2/c, 0)/(c-1))/16
                u = smp.tile([P, T], dt.float32, tag="u")
                nc.vector.tensor_scalar_mul(u[:], s1_all[:], 1.0 / CBIAS)
                ci = smp.tile([P, T], dt.int32, tag="ci")
                nc.vector.tensor_copy(ci[:], u[:])
                cf = smp.tile([P, T], dt.float32, tag="cf")
                nc.vector.tensor_copy(cf[:], ci[:])
                cb = smp.tile([P, T], dt.float32, tag="cb")
                nc.vector.tensor_scalar_mul(cb[:], cf[:], CBIAS)
                s1 = smp.tile([P, T], dt.float32, tag="s1")
                nc.vector.tensor_tensor(
                    out=s1[:], in0=s1_all[:], in1=cb[:], op=Alu.subtract
                )
                rc = smp.tile([P, T], dt.float32, tag="rc")
                nc.vector.reciprocal(rc[:], cf[:])
                s1d = smp.tile([P, T], dt.float32, tag="s1d")
                nc.vector.tensor_tensor(out=s1d[:], in0=s1[:], in1=rc[:], op=Alu.mult)
                m2 = smp.tile([P, T], dt.float32, tag="m2")
                nc.vector.tensor_tensor(out=m2[:], in0=s1[:], in1=s1d[:], op=Alu.mult)
                df = smp.tile([P, T], dt.float32, tag="df")
                nc.vector.tensor_tensor(
                    out=df[:], in0=s2_all[:], in1=m2[:], op=Alu.subtract
                )
                nc.vector.tensor_scalar_max(df[:], df[:], 0.0)
                cm1 = smp.tile([P, T], dt.float32, tag="cm1")
                nc.vector.tensor_scalar_add(cm1[:], cf[:], -1.0)
                rcm = smp.tile([P, T], dt.float32, tag="rcm")
                nc.vector.reciprocal(rcm[:], cm1[:])
                dfn = smp.tile([P, T], dt.float32, tag="dfn")
                nc.vector.tensor_tensor(out=dfn[:], in0=df[:], in1=rcm[:], op=Alu.mult)
                stdt = smp.tile([P, T, 1], dt.float32, tag="stdt")
                nc.scalar.activation(
                    stdt[:].rearrange("p t c -> p (t c)"),
                    dfn[:],
                    AF.Sqrt,
                    scale=1.0 / (K * K),
                )
                nc.sync.dma_start(
                    std_d.ap().rearrange("(t p) c -> p t c", p=P), stdt[:]
                )

    nc.compile()
    return nc


def get_program():
    if "nc" not in _PROG_CACHE:
        _PROG_CACHE["nc"] = _build_program()
    return _PROG_CACHE["nc"]


def make_in_map(x3n: np.ndarray, nrm3n: np.ndarray) -> dict:
    """Per-core inputs. x3n, nrm3n: (3, N) float32."""
    x = np.ascontiguousarray(x3n, dtype=np.float32)          # (3, N)
    xyz = np.ascontiguousarray(x.T)                          # (N, 3)
    nrm = np.ascontiguousarray(np.asarray(nrm3n, np.float32).T)
    sq = (x * x).sum(axis=0, dtype=np.float32)               # (N,)
    ones = np.ones((N,), np.float32)
    rhs5 = np.ascontiguousarray(np.stack([x[0], x[1], x[2], ones, sq]))
    lhsT5 = np.ascontiguousarray(
        np.stack([2 * x[0], 2 * x[1], 2 * x[2], -sq, -ones])
    )
    eye = np.eye(P, dtype=np.float32)
    negpad = np.zeros((P, 896), np.float32)
    negpad[:, 384:512] = np.float32(DIAG_NEG) * eye
    iota = np.tile(np.arange(N, dtype=np.uint32), (P, 1))
    masks = np.empty((P, 18), np.uint32)
    masks[:, 0] = 0xFFFFF000
    masks[:, 1:17] = 0x00000FFF
    masks[:, 17] = 0x00001000
    ones1 = np.ones((1, P), np.float32)
    fbias = np.full((P, 1), CBIAS, np.float32)
    return {
        "lhsT5": lhsT5,
        "rhs5": rhs5,
        "xyz": xyz,
        "nrm": nrm,
        "eye": eye,
        "negpad": negpad,
        "iota": iota,
        "masks": masks,
        "ones1": ones1,
        "fbias": fbias,
    }


def combine(std_vecs: list) -> np.ndarray:
    """std_vecs: 8 arrays (N,) — cores 0-3 ori batches, 4-7 adv batches."""
    dists = []
    for b in range(4):
        diff = (
            std_vecs[b].astype(np.float64)
            - std_vecs[4 + b].astype(np.float64)
            + 1e-6
        )
        dists.append(np.sqrt((diff * diff).sum()))
    return np.asarray(np.mean(dists), dtype=np.float32)


def kernel(ori_data, adv_data, ori_normal):
    from concourse.bass_utils import run_bass_kernel_spmd

    nc = get_program()
    in_maps = []
    for cloud in (ori_data, adv_data):
        for b in range(4):
            in_maps.append(make_in_map(cloud[b], ori_normal[b]))
    res = run_bass_kernel_spmd(nc, in_maps, core_ids=list(range(8)))
    std_vecs = [r["std"][:, 0] for r in res.results]
    return combine(std_vecs)
